# revision 1
# baseline (speedup 1.0000x reference)
"""DVAE GNN message-passing kernel for 8 Trainium2 NeuronCores.

Data parallel over batch B=2048 -> 256 graphs/core. Each core runs the full
20-step topological scan with all weights replicated.

Math (per sample b, step v in 0..19, Hfwd starts at 0):
  gated_u = sigmoid(Wg @ [H_u, e_u] + bg) * (Wm @ [H_u, e_u])
  Hin_v   = sum_u adj[b,u,v] * gated_u          (u >= v rows of Hfwd are 0,
            so gated_u there is a constant c_u)
  H_v     = GRUCell(x_v, Hin_v)
  mu,lv   = W1 @ H_19 + b1, W2 @ H_19 + b2

Device layout: batch-major activations [128b, feat]; matmuls run with the
activation (transposed via PE) as the stationary operand and weights moving,
so outputs land batch-major in PSUM. Biases and the vertex-id one-hot
contributions are folded into the matmuls via ones-rows / one-hot k-chunks.
The adj-weighted message sum runs as fused per-partition-scalar MACs
(scalar_tensor_tensor) split across DVE (batch tile 0) and GPSIMD (tile 1);
the constant part (u >= v) is a real matmul over the u axis seeding the
accumulator in PSUM.
"""

import sys
import numpy as np

for _p in ("/opt/trn_rl_repo",):
    if _p not in sys.path:
        sys.path.insert(0, _p)

B, MAXN, NVT, HS, NZ = 2048, 20, 26, 501, 56
HS2 = HS + 1                  # 502: fp32r needs even innermost free counts
NVT_EFF = NVT + MAXN          # 46
XDIM = NVT_EFF + 1            # 47
NCORES = 8
BS = B // NCORES              # 256 samples per core
G3 = 3 * HS                   # 1503
RZ = 2 * HS                   # 1002

# k-chunking of the augmented hidden axis (501 rows of H^T + ones row)
CH = [(0, 128), (128, 128), (256, 128), (384, 118)]  # covers 0..501 inclusive
# gated-side chunks: + vid one-hot rows appended (total 522 rows)
CHG = [(0, 128), (128, 128), (256, 128), (384, 128), (512, 10)]
CHH = [(0, 128), (128, 128), (256, 128), (384, 128), (512, 10)]  # H^T tile shapes
# transpose source column ranges (chunk3 includes the ones column at HS)
TCH = [(0, 128), (128, 128), (256, 128), (384, 118)]

MM_DTYPE = "f32r"  # "f32r" (1 cyc/row, tf32-ish) | "f32" (4 cyc/row, exact)
USE_GPSIMD = True  # False: route all elementwise TT ops to DVE
ABLATE_ROWDMA = False  # drop per-step ones/vid row DMAs (timing experiment)
ABLATE_CHAIN = False   # drop mask-sum chains (timing experiment)
ABLATE_GATED = False   # drop gated Z/M matmuls + G production (timing)
ABLATE_GRUMM = False   # drop rz/hn/in matmuls (timing)
REPEAT = 1             # repeat the whole computation in-NEFF (timing experiment)
CHAIN2OP = True        # chain as tsmul+TTadd instead of fused stt
PSUM_UNIFIED = False   # one shared 8-buf psum pool instead of 3 pools
WORK_BUFS = 1          # bufs for r/z/tmp/n work tiles


def _pack_layout():
    """Column layout (fp32 elements) of the single packed static tensor.

    Returns (entries, ncols); entries: name -> (row0, nrows, col0, ncols).
    All matmul-consumed slices start at partition 0 or 64.
    """
    ents = {}
    col = 0

    def put(name, row0, nrows, ncols):
        nonlocal col
        ents[name] = (row0, nrows, col, ncols)
        col += ncols

    put("pk", 0, 84, MAXN * BS)          # rows 0:48 X^T+ones, 64:84 adjT masked
    for i, (o, s) in enumerate(CH):
        put(f"wrzh{i}", 0, s, 2 * HS2)
    for i, (o, s) in enumerate(CH):
        put(f"whn{i}", 0, s, HS2)
    put("wrzx", 0, XDIM + 1, 2 * HS2)
    put("wxnc", 0, 84, HS2)              # rows 0:48 W_in^T+bias, 64:84 C
    for i, (o, s) in enumerate(CH):
        put(f"wg{i}", 0, s, HS2)
    put("wgv", 0, MAXN, HS2)
    for i, (o, s) in enumerate(CH):
        put(f"wm{i}", 0, s, HS2)
    put("wmv", 0, MAXN, HS2)
    put("eye20", 0, MAXN, MAXN)
    for i, (o, s) in enumerate(CH):
        put(f"w12{i}", 0, s, 2 * NZ)
    put("adjg0", 0, 128, MAXN * MAXN)
    put("adjg1", 0, 128, MAXN * MAXN)
    put("ident", 0, 128, 128)
    return ents, col


_PROG = None  # cached Bass program


def _build_program():
    import concourse.bass as bass
    import concourse.tile as tile
    from concourse import bacc, mybir

    f32 = mybir.dt.float32
    f32r = mybir.dt.float32r
    mdt = {"f32r": f32r, "f32": f32, "bf16": mybir.dt.bfloat16}[MM_DTYPE]
    AF = mybir.ActivationFunctionType
    OP = mybir.AluOpType

    nc = bacc.Bacc("TRN2", target_bir_lowering=False, debug=False)

    def din(name, shape, dt=None):
        return nc.dram_tensor(name, shape, dt or mdt, kind="ExternalInput").ap()

    ents, ncols = _pack_layout()
    d_wpack = din("wpack", [128, ncols])

    d_out = nc.dram_tensor("out", [BS, 2 * NZ], f32, kind="ExternalOutput").ap()

    def mm(out, lhsT, rhs, start, stop):
        nc.tensor.matmul(out, lhsT, rhs, start=start, stop=stop)

    with tile.TileContext(nc) as tc:
        with (
            tc.tile_pool(name="statics", bufs=1) as sp,
            tc.tile_pool(name="gstore", bufs=2 * (MAXN - 1)) as gp,
            tc.tile_pool(name="hint", bufs=2) as hip,
            tc.tile_pool(name="ht", bufs=2) as htp,
            tc.tile_pool(name="work1", bufs=WORK_BUFS) as wp1,
            tc.tile_pool(name="work2", bufs=2) as wp2,
            tc.tile_pool(name="pp_rz", bufs=(8 if PSUM_UNIFIED else 3),
                         space="PSUM") as pp_rz,
            tc.tile_pool(name="pp_tps", bufs=3, space="PSUM") as _pp_tps,
            tc.tile_pool(name="pp_hn", bufs=2, space="PSUM") as _pp_hn,
        ):
            pp_tps = pp_rz if PSUM_UNIFIED else _pp_tps
            pp_hn = pp_rz if PSUM_UNIFIED else _pp_hn
            # ---- one packed static load: a single DMA -> a single wait sem ----
            WPACK = sp.tile([128, ncols], mdt, tag="wpack", name="wpack")
            nc.sync.dma_start(WPACK[:, :], d_wpack)

            def sl(name, dt=None):
                r0, nr, c0, ncl = ents[name]
                ap = WPACK[r0:r0 + nr, c0:c0 + ncl]
                return ap.bitcast(dt) if dt else ap

            PK = sl("pk")
            WRZH = [sl(f"wrzh{i}") for i in range(4)]
            WHN = [sl(f"whn{i}") for i in range(4)]
            WRZX = sl("wrzx")
            WXNC = sl("wxnc")
            WG = [sl(f"wg{i}") for i in range(4)]
            WM = [sl(f"wm{i}") for i in range(4)]
            WGV, WMV, EYE = sl("wgv"), sl("wmv"), sl("eye20")
            W12 = [sl(f"w12{i}") for i in range(4)]
            IDN = sl("ident", f32)
            bf16 = mybir.dt.bfloat16
            ADJG = [sl(f"adjg{t}", f32) for t in range(2)]

            # G storage: gated vectors per (vertex, batch-tile), bf16 so the
            # message chains run in the DVE 2x mode
            Gt = [[gp.tile([128, HS2], bf16, tag="g", name=f"g{_u}_{_t}")
                   for _t in range(2)] for _u in range(MAXN - 1)]

            gpe = nc.gpsimd if USE_GPSIMD else nc.vector

            HT_final = None
            for _rep in range(REPEAT):
              for v in range(MAXN):
                  # ---- message input Hin_v, batch-major, per batch tile ----
                  # acc column HS holds 1.0 (ones row of Hinaug^T after transpose)
                  acc = []
                  for t in range(2):
                      dps = pp_tps.tile([128, 512], f32, tag=("rz" if PSUM_UNIFIED else "tps"), name=f"dps{v}_{t}")
                      # constant part: sum_{u>=v} adj[b,u,v] * C[u]
                      mm(dps[:, :HS2], PK[64:84, v * BS + t * 128:v * BS + (t + 1) * 128],
                         WXNC[64:84, :], start=True, stop=True)
                      a = wp2.tile([128, HS2], f32, tag=f"acc{t}", name=f"acc{v}_{t}")
                      if v == 0 or ABLATE_CHAIN:
                          nc.scalar.copy(a[:, :], dps[:, :HS2])
                      else:
                          # bf16 chain on DVE: 4x mul, 2x fused MACs, f32 tail
                          ab = wp1.tile([128, HS2], bf16, tag=f"accb{t}",
                                        name=f"accb{v}_{t}")
                          ab2 = (wp1.tile([128, HS2], bf16, tag=f"accc{t}",
                                          name=f"accc{v}_{t}") if CHAIN2OP else None)
                          for u in range(v):
                              sc = ADJG[t][:, u * MAXN + v:u * MAXN + v + 1]
                              if u == 0:
                                  nc.vector.tensor_scalar_mul(ab[:, :], Gt[u][t][:, :], sc)
                              elif CHAIN2OP:
                                  nc.vector.tensor_scalar_mul(ab2[:, :], Gt[u][t][:, :], sc)
                                  nc.vector.tensor_tensor(ab[:, :], ab[:, :], ab2[:, :],
                                                          OP.add)
                              else:
                                  nc.vector.scalar_tensor_tensor(
                                      ab[:, :], Gt[u][t][:, :], sc, ab[:, :],
                                      OP.mult, OP.add)
                          nc.vector.tensor_tensor(a[:, :], ab[:, :], dps[:, :HS2], OP.add)
                      nc.gpsimd.memset(a[:, HS:HS2], 1.0)   # ones col -> bias row
                      acc.append(a)

                  # ---- transpose Hin -> Hinaug^T chunk-pair tiles ----
                  # pair tile p holds chunks 2p (cols 0:256) and 2p+1 (cols 256:512)
                  HINT = [hip.tile([128, 512], mdt, tag=f"hint{p}", name=f"hint{v}_{p}")
                          for p in range(2)]
                  for p in range(2):
                      tp = pp_tps.tile([128, 512], f32, tag=("rz" if PSUM_UNIFIED else "tps"), name=f"tpi{v}_{p}")
                      for j in range(2):
                          i = 2 * p + j
                          o, w = TCH[i]
                          for t in range(2):
                              nc.tensor.transpose(
                                  tp[:w, j * 256 + t * 128:j * 256 + (t + 1) * 128],
                                  acc[t][:, o:o + w], IDN[:, :])
                      nc.scalar.copy(HINT[p][:, :], tp[:, :])

                  def hsl(i, t):
                      return HINT[i // 2][0:TCH[i][1], (i % 2) * 256 + t * 128:
                                          (i % 2) * 256 + (t + 1) * 128]

                  # ---- gate matmuls ----
                  rzp, hnp, inp = [], [], []
                  for t in range(2):
                      xsl = PK[0:XDIM + 1, v * BS + t * 128:v * BS + (t + 1) * 128]
                      if ABLATE_GRUMM:
                          ps0 = pp_rz.tile([128, 512], f32, tag="rz", name=f"rz{v}_{t}_0")
                          mm(ps0[:, :HS2], xsl, WRZX[:, 0:HS2], start=True, stop=True)
                          ps1 = pp_rz.tile([128, 512], f32, tag="rz", name=f"rz{v}_{t}_1")
                          mm(ps1[:, :HS2], xsl, WRZX[:, HS2:2 * HS2], start=True, stop=True)
                          rzp += [ps0, ps1]
                          hnx = pp_hn.tile([128, 512], f32, tag=("rz" if PSUM_UNIFIED else "hn"), name=f"hn{v}_{t}")
                          mm(hnx[:, :HS2], xsl, WXNC[0:XDIM + 1, :], start=True, stop=True)
                          hnp.append(hnx)
                          ipx = pp_tps.tile([128, 512], f32, tag=("rz" if PSUM_UNIFIED else "tps"), name=f"in{v}_{t}")
                          mm(ipx[:, :HS2], xsl, WXNC[0:XDIM + 1, :], start=True, stop=True)
                          inp.append(ipx)
                          continue
                      for j in range(2):  # r and z halves
                          ps = pp_rz.tile([128, 512], f32, tag="rz", name=f"rz{v}_{t}_{j}")
                          for i in range(4):
                              mm(ps[:, :HS2], hsl(i, t),
                                 WRZH[i][:, j * HS2:(j + 1) * HS2], start=(i == 0), stop=False)
                          mm(ps[:, :HS2], xsl, WRZX[:, j * HS2:(j + 1) * HS2],
                             start=False, stop=True)
                          rzp.append(ps)
                      hn = pp_hn.tile([128, 512], f32, tag=("rz" if PSUM_UNIFIED else "hn"), name=f"hn{v}_{t}")
                      for i in range(4):
                          mm(hn[:, :HS2], hsl(i, t), WHN[i][:, :],
                             start=(i == 0), stop=(i == 3))
                      hnp.append(hn)
                      ip = pp_tps.tile([128, 512], f32, tag=("rz" if PSUM_UNIFIED else "tps"), name=f"in{v}_{t}")
                      mm(ip[:, :HS2], xsl, WXNC[0:XDIM + 1, :], start=True, stop=True)
                      inp.append(ip)

                  # ---- GRU elementwise ----
                  hb = []
                  for t in range(2):
                      r = wp1.tile([128, HS2], f32, tag=f"r{t}", name=f"r{v}_{t}")
                      z = wp1.tile([128, HS2], f32, tag=f"z{t}", name=f"z{v}_{t}")
                      nc.scalar.activation(r[:, :HS], rzp[2 * t][:, :HS], AF.Sigmoid)
                      nc.scalar.activation(z[:, :HS], rzp[2 * t + 1][:, :HS], AF.Sigmoid)
                      tmp = wp1.tile([128, HS], f32, tag=f"tmp{t}", name=f"tmp{v}_{t}")
                      nc.vector.tensor_tensor(tmp[:, :], r[:, :HS], hnp[t][:, :HS], OP.mult)
                      nc.vector.tensor_tensor(tmp[:, :], tmp[:, :], inp[t][:, :HS], OP.add)
                      n = wp1.tile([128, HS], f32, tag=f"n{t}", name=f"n{v}_{t}")
                      nc.scalar.activation(n[:, :], tmp[:, :], AF.Tanh)
                      d = wp1.tile([128, HS], f32, tag=f"tmp{t}", name=f"d{v}_{t}")
                      gpe.tensor_sub(d[:, :], acc[t][:, :HS], n[:, :])
                      h = wp2.tile([128, HS2], f32, tag=f"h{t}", name=f"h{v}_{t}")
                      gpe.tensor_tensor(h[:, :HS], d[:, :], z[:, :HS], OP.mult)
                      gpe.tensor_tensor(h[:, :HS], h[:, :HS], n[:, :], OP.add)
                      nc.gpsimd.memset(h[:, HS:HS2], 1.0)   # ones col -> bg row
                      hb.append(h)

                  # ---- transpose H -> Haug^T chunk-pair tiles ----
                  HT = [htp.tile([128, 512], mdt, tag=f"ht{p}", name=f"ht{v}_{p}")
                        for p in range(2)]
                  for p in range(2):
                      tp = pp_tps.tile([128, 512], f32, tag=("rz" if PSUM_UNIFIED else "tps"), name=f"tph{v}_{p}")
                      for j in range(2):
                          i = 2 * p + j
                          o, w = TCH[i]
                          for t in range(2):
                              nc.tensor.transpose(
                                  tp[:w, j * 256 + t * 128:j * 256 + (t + 1) * 128],
                                  hb[t][:, o:o + w], IDN[:, :])
                      nc.scalar.copy(HT[p][:, :], tp[:, :])

                  def htl(i, t):
                      return HT[i // 2][0:TCH[i][1], (i % 2) * 256 + t * 128:
                                        (i % 2) * 256 + (t + 1) * 128]

                  if v < MAXN - 1 and not ABLATE_GATED:
                      # ---- gated message for this vertex ----
                      # vid one-hot contribution: broadcast-selected column of EYE
                      vsel = EYE[:, v:v + 1].broadcast_to([MAXN, 128])
                      for t in range(2):
                          zp = pp_rz.tile([128, 512], f32, tag="rz", name=f"zp{v}_{t}")
                          mp = pp_rz.tile([128, 512], f32, tag="rz", name=f"mp{v}_{t}")
                          for i in range(4):
                              mm(zp[:, :HS2], htl(i, t), WG[i][:, :],
                                 start=(i == 0), stop=False)
                          mm(zp[:, :HS2], vsel, WGV[:, :], start=False, stop=True)
                          for i in range(4):
                              mm(mp[:, :HS2], htl(i, t), WM[i][:, :],
                                 start=(i == 0), stop=False)
                          mm(mp[:, :HS2], vsel, WMV[:, :], start=False, stop=True)
                          sg = wp1.tile([128, HS2], f32, tag=f"r{t}", name=f"sg{v}_{t}")
                          nc.scalar.activation(sg[:, :], zp[:, :HS2], AF.Sigmoid)
                          mb = wp1.tile([128, HS2], f32, tag=f"z{t}", name=f"mb{v}_{t}")
                          nc.scalar.copy(mb[:, :], mp[:, :HS2])
                          gpe.tensor_tensor(Gt[v][t][:, :], sg[:, :], mb[:, :], OP.mult)
                  if v == MAXN - 1:
                      HT_final = HT

            # ---- readout ----
            HTf = HT_final
            for t in range(2):
                op = pp_hn.tile([128, 512], f32, tag=("rz" if PSUM_UNIFIED else "hn"), name=f"op{t}")
                for i in range(4):
                    ksl = HTf[i // 2][0:TCH[i][1], (i % 2) * 256 + t * 128:
                                      (i % 2) * 256 + (t + 1) * 128]
                    mm(op[:, :2 * NZ], ksl,
                       W12[i][:, :], start=(i == 0), stop=(i == 3))
                ob = wp1.tile([128, 2 * NZ], f32, tag=f"ob{t}", name=f"ob{t}")
                nc.scalar.copy(ob[:, :], op[:, :2 * NZ])
                nc.sync.dma_start(d_out[t * 128:(t + 1) * 128, :], ob[:, :])

    nc.compile()
    return nc


def _host_prep(types, feats, adj, Wg, bg, Wm, W_ih, b_ih, W_hh, b_hh, W1, b1, W2, b2):
    """Build per-core input maps (numpy only)."""
    f = np.float32
    types = np.asarray(types).astype(np.int64)
    feats = np.asarray(feats, dtype=f)
    adj = np.asarray(adj, dtype=f)
    Wg, bg, Wm = np.asarray(Wg, f), np.asarray(bg, f), np.asarray(Wm, f)
    W_ih, b_ih = np.asarray(W_ih, f), np.asarray(b_ih, f)
    W_hh, b_hh = np.asarray(W_hh, f), np.asarray(b_hh, f)
    W1, b1 = np.asarray(W1, f), np.asarray(b1, f)
    W2, b2 = np.asarray(W2, f), np.asarray(b2, f)

    bsz = types.shape[0]
    ncore = NCORES
    bs = bsz // ncore

    # X^T with ones row: [48, MAXN*bs] per core
    X = np.zeros((bsz, MAXN, XDIM + 1), dtype=f)
    onehot = np.eye(NVT_EFF, dtype=f)[types.reshape(-1) % NVT_EFF]
    X[:, :, :NVT_EFF] = onehot.reshape(bsz, MAXN, NVT_EFF)
    X[:, :, NVT_EFF] = feats
    X[:, :, XDIM] = 1.0

    # constant gated vectors c_u for zero hidden state
    zg = 1.0 / (1.0 + np.exp(-(bg[None, :] + Wg[:, HS:].T)))   # [20, 501]
    C = (zg * Wm[:, HS:].T).astype(f)

    def aug(wT, brow):
        return np.concatenate([wT, brow[None, :]], axis=0).astype(f)

    def pad_rz(a):          # [s, 1002] -> [s, 1004] with per-gate 502 halves
        o = np.zeros((a.shape[0], 2 * HS2), dtype=f)
        o[:, :HS] = a[:, :HS]
        o[:, HS2:HS2 + HS] = a[:, HS:]
        return o

    def pad_h(a):           # [s, 501] -> [s, 502]
        o = np.zeros((a.shape[0], HS2), dtype=f)
        o[:, :HS] = a
        return o

    wrzh = pad_rz(aug(W_hh[:RZ].T, b_hh[:RZ]))
    whn = pad_h(aug(W_hh[RZ:].T, b_hh[RZ:]))
    wrzx = pad_rz(aug(W_ih[:RZ].T, b_ih[:RZ]))
    wxnc = np.zeros((84, HS2), dtype=f)
    wxnc[:XDIM + 1] = pad_h(aug(W_ih[RZ:].T, b_ih[RZ:]))
    wxnc[64:84] = pad_h(C)
    wg = pad_h(np.concatenate([Wg[:, :HS].T, bg[None, :]], axis=0).astype(f))
    wgv = pad_h(np.ascontiguousarray(Wg[:, HS:].T))
    wm = pad_h(np.concatenate([Wm[:, :HS].T, np.zeros((1, HS), f)], axis=0))
    wmv = pad_h(np.ascontiguousarray(Wm[:, HS:].T))
    eye20 = np.eye(MAXN, dtype=f)
    w12 = np.concatenate([np.concatenate([W1.T, W2.T], axis=1),
                          np.concatenate([b1, b2])[None, :]], axis=0).astype(f)
    ident = np.eye(128, dtype=f)

    ents, ncols = _pack_layout()

    def place(pack, name, arr):
        r0, nr, c0, ncl = ents[name]
        assert arr.shape == (nr, ncl), (name, arr.shape, (nr, ncl))
        pack[r0:r0 + nr, c0:c0 + ncl] = arr

    umask = (np.arange(MAXN)[:, None] >= np.arange(MAXN)[None, :]).astype(f)

    in_maps = []
    for c in range(ncore):
        sl = slice(c * bs, (c + 1) * bs)
        Xc = X[sl]                                    # [bs, 20, 48]
        xt = Xc.transpose(2, 1, 0).reshape(XDIM + 1, MAXN * bs)
        adjc = adj[sl]                                # [bs, 20, 20]
        # adjT[u, v*bs+b] = adj[b,u,v], zeroed where u < v (only u>=v used)
        adjm = adjc.transpose(1, 2, 0) * umask[:, :, None]
        pk = np.zeros((84, MAXN * bs), dtype=f)
        pk[:XDIM + 1] = xt
        pk[64:84] = adjm.reshape(MAXN, MAXN * bs)

        pack = np.zeros((128, ncols), dtype=f)
        place(pack, "pk", pk)
        for i, (o, s) in enumerate(CH):
            place(pack, f"wrzh{i}", wrzh[o:o + s])
            place(pack, f"whn{i}", whn[o:o + s])
            place(pack, f"w12{i}", w12[o:o + s])
        place(pack, "wrzx", wrzx)
        place(pack, "wxnc", wxnc)
        for i, (o, s) in enumerate(CH):
            place(pack, f"wg{i}", wg[o:o + s])
            place(pack, f"wm{i}", wm[o:o + s])
        place(pack, "wgv", wgv)
        place(pack, "wmv", wmv)
        place(pack, "eye20", eye20)
        adjg = adjc.reshape(bs, MAXN * MAXN)
        place(pack, "adjg0", adjg[:128])
        place(pack, "adjg1", adjg[128:])
        place(pack, "ident", ident)
        in_maps.append(dict(wpack=pack))
    return in_maps


def _get_prog():
    global _PROG
    if _PROG is None:
        _PROG = _build_program()
    return _PROG


def kernel(**inputs):
    from concourse.bass_utils import run_bass_kernel_spmd
    nc = _get_prog()
    in_maps = _host_prep(**inputs)
    res = run_bass_kernel_spmd(nc, in_maps, core_ids=list(range(NCORES)))
    out = np.concatenate([r["out"] for r in res.results], axis=0)
    mu = np.ascontiguousarray(out[:, :NZ])
    logvar = np.ascontiguousarray(out[:, NZ:])
    return mu, logvar



# revision 16
# speedup vs baseline: 1.4330x; 1.4330x over previous
"""DVAE GNN message-passing kernel for 8 Trainium2 NeuronCores.

Data parallel over batch B=2048 -> 256 graphs/core (2 tiles of 128). Each core
runs the full 20-step topological scan with all weights replicated.

Math (per sample b, step v in 0..19, Hfwd starts at 0):
  gated_u = sigmoid(Wg @ [H_u, e_u] + bg) * (Wm @ [H_u, e_u])
  Hin_v   = sum_u adj[b,u,v] * gated_u      (u >= v rows give a constant
            contribution, precomputed on host and DMA'd as the slot seed)
  H_v     = GRUCell(x_v, Hin_v)
  mu,lv   = W1 @ H_19 + b1, W2 @ H_19 + b2

Device design notes:
  - Batch-major activations [128b, feat]; matmuls use the (transposed)
    activation as stationary and weights as moving, outputs land batch-major.
  - Per-step message inputs live in 40 persistent bf16 "slot" accumulators
    (one per (tile, step), ones-column at col HS baked in by the host).
    Each gated vector G_v is scattered into future slots with fused
    per-partition-scalar MACs on DVE (bf16 2x) / Pool (deferred queue).
  - All PE transposes run with a bf16 identity (1 cyc/row); Hin^T / H^T
    tiles are bf16 stationaries against f32r moving weights.
  - Emission order interleaves the two batch tiles so the PE stays fed
    during the GRU elementwise phase (pstate ramp).
"""

import sys
import numpy as np

for _p in ("/opt/trn_rl_repo",):
    if _p not in sys.path:
        sys.path.insert(0, _p)

B, MAXN, NVT, HS, NZ = 2048, 20, 26, 501, 56
HS2 = HS + 1                  # 502 (even innermost counts for fp32r)
NVT_EFF = NVT + MAXN          # 46
XDIM = NVT_EFF + 1            # 47
XD = XDIM + 1                 # 48: + ones row
NCORES = 8
BS = B // NCORES              # 256 samples per core
RZ = 2 * HS                   # 1002

# k-chunking of the augmented hidden axis (501 rows + ones row = 502)
CH = [(0, 128), (128, 128), (256, 128), (384, 118)]

DVE_SCAT_CAP = 22  # max scatter MACs per step on DVE (rest deferred)


def _packf_layout():
    """Column layout (f32 elements) of the packed f32r static tensor."""
    ents = {}
    col = 0

    def put(name, nrows, ncols):
        nonlocal col
        ents[name] = (nrows, col, ncols)
        col += ncols

    put("xub", XD, MAXN * BS)            # X^T + ones row
    for i, (o, s) in enumerate(CH):
        put(f"wrzh{i}", s, 2 * HS2)
    for i, (o, s) in enumerate(CH):
        put(f"whn{i}", s, HS2)
    put("wrzx", XD, 2 * HS2)
    put("wxn", XD, HS2)
    for i, (o, s) in enumerate(CH):
        put(f"wg{i}", s, HS2)
    put("wgv", MAXN, HS2)
    for i, (o, s) in enumerate(CH):
        put(f"wm{i}", s, HS2)
    put("wmv", MAXN, HS2)
    put("eye20", MAXN, MAXN)
    for i, (o, s) in enumerate(CH):
        put(f"w12{i}", s, 2 * NZ)
    put("adjg0", 128, MAXN * MAXN)
    put("adjg1", 128, MAXN * MAXN)
    return ents, col


NBCOLS = 40 * HS2 + 128  # bf16 pack: 40 slots + identity


_PROG = None  # cached Bass program


def _build_program():
    import concourse.bass as bass
    import concourse.tile as tile
    from concourse import bacc, mybir

    f32 = mybir.dt.float32
    f32r = mybir.dt.float32r
    bf16 = mybir.dt.bfloat16
    AF = mybir.ActivationFunctionType
    OP = mybir.AluOpType

    nc = bacc.Bacc("TRN2", target_bir_lowering=False, debug=False)

    ents, ncolsf = _packf_layout()
    d_wpack = nc.dram_tensor("wpack", [128, ncolsf], f32r,
                             kind="ExternalInput").ap()
    d_bpack = nc.dram_tensor("bpack", [128, NBCOLS], bf16,
                             kind="ExternalInput").ap()
    d_out = nc.dram_tensor("out", [BS, 2 * NZ], f32, kind="ExternalOutput").ap()

    def mm(out, lhsT, rhs, start, stop):
        nc.tensor.matmul(out, lhsT, rhs, start=start, stop=stop)

    with tile.TileContext(nc) as tc:
        with (
            tc.tile_pool(name="statics", bufs=1) as sp,
            tc.tile_pool(name="gstore", bufs=1) as gp,
            tc.tile_pool(name="sb", bufs=1) as wp,
            tc.tile_pool(name="pp", bufs=1, space="PSUM") as pp,
        ):
            WPACK = sp.tile([128, ncolsf], f32r, tag="wpack", name="wpack")
            nc.sync.dma_start(WPACK[:, :], d_wpack)
            BPACK = sp.tile([128, NBCOLS], bf16, tag="bpack", name="bpack")
            nc.sync.dma_start(BPACK[:, :], d_bpack)

            def sl(name, dt=None):
                nr, c0, ncl = ents[name]
                ap = WPACK[0:nr, c0:c0 + ncl]
                return ap.bitcast(dt) if dt else ap

            XUB = sl("xub")
            WRZH = [sl(f"wrzh{i}") for i in range(4)]
            WHN = [sl(f"whn{i}") for i in range(4)]
            WRZX = sl("wrzx")
            WXN = sl("wxn")
            WG = [sl(f"wg{i}") for i in range(4)]
            WM = [sl(f"wm{i}") for i in range(4)]
            WGV, WMV, EYE = sl("wgv"), sl("wmv"), sl("eye20")
            W12 = [sl(f"w12{i}") for i in range(4)]
            ADJG = [sl(f"adjg{t}", f32) for t in range(2)]

            def SLOT(t, w):
                c = (t * MAXN + w) * HS2
                return BPACK[:, c:c + HS2]

            IDB = BPACK[:, 40 * HS2:40 * HS2 + 128]

            # G storage: one bf16 tile per (vertex, batch-tile)
            Gt = [[gp.tile([128, HS2], bf16, tag=f"g{_u}_{_t}",
                           name=f"g{_u}_{_t}")
                   for _t in range(2)] for _u in range(MAXN - 1)]

            # SBUF work tiles (tags give fixed buffers; bufs chosen for
            # cross-step pipelining where needed)
            def wtile(tag, shape, dt, bufs, name):
                return wp.tile(shape, dt, tag=tag, bufs=bufs, name=name)

            # psum tiles: all padded to one full 2KB bank
            def ptile(tag, dt, bufs, name):
                pad = [128, 512] if dt == f32 else [128, 1024]
                return pp.tile([128, 512], dt, tag=tag, bufs=bufs,
                               padded_shape=pad, name=name)

            # ---- scatter queue state (python-side scheduling) ----
            pend = []  # list of (w, u, t) pairs not yet emitted

            def emit_mac(eng, u, w, t):
                sc = ADJG[t][:, u * MAXN + w:u * MAXN + w + 1]
                eng.scalar_tensor_tensor(SLOT(t, w), Gt[u][t][:, :], sc,
                                         SLOT(t, w), OP.mult, OP.add)

            hint = {}
            ht = {}
            gates = {}

            def phase_A(v, t):
                """xseeds, transpose acc, hint copy, rz/hn matmuls for tile t."""
                xsl = XUB[:, v * BS + t * 128:v * BS + (t + 1) * 128]
                rz0 = ptile(f"rz0", f32, 1, f"rz0_{v}_{t}")
                rz1 = ptile(f"rz1", f32, 1, f"rz1_{v}_{t}")
                inp = ptile(f"inp", f32, 1, f"inp_{v}_{t}")
                hnp = ptile(f"hnp", f32, 1, f"hnp_{v}_{t}")
                gates[t] = (rz0, rz1, inp, hnp)
                mm(rz0[:, :HS2], xsl, WRZX[:, 0:HS2], start=True, stop=False)
                mm(rz1[:, :HS2], xsl, WRZX[:, HS2:2 * HS2], start=True,
                   stop=False)
                mm(inp[:, :HS2], xsl, WXN[:, :], start=True, stop=True)
                # transpose Hin (slot v) -> psum, then copy to sbuf (f32r)
                tp = ptile("tp", bf16, 2, f"tpa_{v}_{t}")
                acc = SLOT(t, v)
                for i, (o, w) in enumerate(CH):
                    nc.tensor.transpose(tp[0:w, i * 128:(i + 1) * 128],
                                        acc[:, o:o + w], IDB)
                hi = wtile(f"hint{t}", [128, 512], f32r, 2, f"hint_{v}_{t}")
                hint[t] = hi
                nc.scalar.copy(hi[:, :], tp[:, :512])

                def hc(i):
                    return hi[0:CH[i][1], i * 128:(i + 1) * 128]

                for i in range(4):
                    mm(rz0[:, :HS2], hc(i), WRZH[i][:, 0:HS2],
                       start=False, stop=(i == 3))
                for i in range(4):
                    mm(rz1[:, :HS2], hc(i), WRZH[i][:, HS2:2 * HS2],
                       start=False, stop=(i == 3))
                for i in range(4):
                    mm(hnp[:, :HS2], hc(i), WHN[i][:, :],
                       start=(i == 0), stop=(i == 3))

            def gru_front(v, t):
                """sigmoids + tanh input for tile t (Act/DVE)."""
                rz0, rz1, inp, hnp = gates[t]
                r = wtile(f"r{t}", [128, HS2], bf16, 1, f"r_{v}_{t}")
                z = wtile(f"z{t}", [128, HS2], bf16, 1, f"z_{v}_{t}")
                nc.scalar.activation(r[:, :HS], rz0[:, :HS], AF.Sigmoid)
                nc.scalar.activation(z[:, :HS], rz1[:, :HS], AF.Sigmoid)
                tmp = wtile(f"tmp{t}", [128, HS], f32, 1, f"tmp_{v}_{t}")
                nc.vector.tensor_tensor(tmp[:, :], r[:, :HS], hnp[:, :HS],
                                        OP.mult)
                nc.vector.tensor_tensor(tmp[:, :], tmp[:, :], inp[:, :HS],
                                        OP.add)
                return z, tmp

            def gru_back(v, t, z, n):
                """post-tanh GRU ops -> h tile (bf16); t0 on DVE, t1 on Pool
                (all-SBUF operands, keeps DVE free for the scatter MACs)."""
                eng = nc.vector if t == 0 else nc.gpsimd
                d = wtile(f"d{t}", [128, HS], bf16, 1, f"d_{v}_{t}")
                eng.tensor_tensor(d[:, :], SLOT(t, v)[:, :HS], n[:, :],
                                  OP.subtract)
                e = wtile(f"e{t}", [128, HS], bf16, 1, f"e_{v}_{t}")
                eng.tensor_tensor(e[:, :], d[:, :], z[:, :HS], OP.mult)
                h = wtile(f"h{t}", [128, HS2], bf16, 2, f"h_{v}_{t}")
                nc.gpsimd.memset(h[:, HS:HS2], 1.0)
                eng.tensor_tensor(h[:, :HS], e[:, :], n[:, :], OP.add)
                return h

            def phase_B_pe(v, t, h):
                """transpose h, ht copy, zp/mp matmuls for tile t."""
                tp = ptile("tp", bf16, 2, f"tpb_{v}_{t}")
                for i, (o, w) in enumerate(CH):
                    nc.tensor.transpose(tp[0:w, i * 128:(i + 1) * 128],
                                        h[:, o:o + w], IDB)
                hb = wtile(f"ht{t}", [128, 512], f32r, 2, f"ht_{v}_{t}")
                ht[t] = hb
                nc.scalar.copy(hb[:, :], tp[:, :512])

                def hc(i):
                    return hb[0:CH[i][1], i * 128:(i + 1) * 128]

                if v < MAXN - 1:
                    vsel = EYE[:, v:v + 1].broadcast_to([MAXN, 128])
                    zp = ptile("zp", f32, 1, f"zp_{v}_{t}")
                    mp = ptile("mp", f32, 1, f"mp_{v}_{t}")
                    mm(zp[:, :HS2], vsel, WGV[:, :], start=True, stop=False)
                    for i in range(4):
                        mm(zp[:, :HS2], hc(i), WG[i][:, :],
                           start=False, stop=(i == 3))
                    mm(mp[:, :HS2], vsel, WMV[:, :], start=True, stop=False)
                    for i in range(4):
                        mm(mp[:, :HS2], hc(i), WM[i][:, :],
                           start=False, stop=(i == 3))
                    return zp, mp
                return None, None

            def make_G(v, t, zp, mp):
                sg = wtile(f"sg{t}", [128, HS2], bf16, 1, f"sg_{v}_{t}")
                nc.scalar.activation(sg[:, :], zp[:, :HS2], AF.Sigmoid)
                nc.vector.tensor_tensor(Gt[v][t][:, :], sg[:, :],
                                        mp[:, :HS2], OP.mult)
                # critical MAC into the next step's slot
                emit_mac(nc.vector, v, v + 1, t)

            # ================= main loop =================
            for v in range(MAXN):
                phase_A(v, 0)
                z0, tmp0 = gru_front(v, 0)
                phase_A(v, 1)
                n0 = wtile("n0", [128, HS], bf16, 1, f"n_{v}_0")
                nc.scalar.activation(n0[:, :], tmp0[:, :], AF.Tanh)
                h0 = gru_back(v, 0, z0, n0)
                z1, tmp1 = gru_front(v, 1)
                zp0, mp0 = phase_B_pe(v, 0, h0)
                n1 = wtile("n1", [128, HS], bf16, 1, f"n_{v}_1")
                nc.scalar.activation(n1[:, :], tmp1[:, :], AF.Tanh)
                h1 = gru_back(v, 1, z1, n1)
                if v < MAXN - 1:
                    make_G(v, 0, zp0, mp0)
                zp1, mp1 = phase_B_pe(v, 1, h1)
                if v < MAXN - 1:
                    make_G(v, 1, zp1, mp1)
                    # queue far-future scatter for this step's G
                    for w in range(v + 2, MAXN):
                        pend.append((w, v, 0))
                        pend.append((w, v, 1))
                    # DVE drain: mandatory (w == v+1 targets) + budget
                    pend.sort()
                    ndve = 0
                    keep = []
                    for (w, u, t) in pend:
                        if w == v + 2 or ndve < DVE_SCAT_CAP:
                            emit_mac(nc.vector, u, w, t)
                            ndve += 1
                        else:
                            keep.append((w, u, t))
                    pend = keep

            # ---- readout from ht tiles of v=19 ----
            for t in range(2):
                op = ptile("zp", f32, 1, f"op_{t}")
                hb = ht[t]
                for i in range(4):
                    mm(op[:, :2 * NZ], hb[0:CH[i][1], i * 128:(i + 1) * 128],
                       W12[i][:, :], start=(i == 0), stop=(i == 3))
                ob = wtile(f"ob{t}", [128, 2 * NZ], f32, 1, f"ob_{t}")
                nc.scalar.copy(ob[:, :], op[:, :2 * NZ])
                nc.sync.dma_start(d_out[t * 128:(t + 1) * 128, :], ob[:, :])

    nc.compile()
    return nc


def _host_prep(types, feats, adj, Wg, bg, Wm, W_ih, b_ih, W_hh, b_hh, W1, b1,
               W2, b2):
    """Build per-core input maps (numpy only)."""
    import ml_dtypes
    bf16 = ml_dtypes.bfloat16
    f = np.float32
    types = np.asarray(types).astype(np.int64)
    feats = np.asarray(feats, dtype=f)
    adj = np.asarray(adj, dtype=f)
    Wg, bg, Wm = np.asarray(Wg, f), np.asarray(bg, f), np.asarray(Wm, f)
    W_ih, b_ih = np.asarray(W_ih, f), np.asarray(b_ih, f)
    W_hh, b_hh = np.asarray(W_hh, f), np.asarray(b_hh, f)
    W1, b1 = np.asarray(W1, f), np.asarray(b1, f)
    W2, b2 = np.asarray(W2, f), np.asarray(b2, f)

    bsz = types.shape[0]
    bs = bsz // NCORES

    # X^T with ones row: [48, MAXN*bs] per core
    X = np.zeros((bsz, MAXN, XD), dtype=f)
    onehot = np.eye(NVT_EFF, dtype=f)[types.reshape(-1) % NVT_EFF]
    X[:, :, :NVT_EFF] = onehot.reshape(bsz, MAXN, NVT_EFF)
    X[:, :, NVT_EFF] = feats
    X[:, :, XDIM] = 1.0

    # constant gated vectors c_u for zero hidden state
    zg = 1.0 / (1.0 + np.exp(-(bg[None, :] + Wg[:, HS:].T)))   # [20, 501]
    C = (zg * Wm[:, HS:].T).astype(f)
    # Hin constant part for every (sample, step): sum_{u>=w} adj[b,u,w]*C[u]
    umask = (np.arange(MAXN)[:, None] >= np.arange(MAXN)[None, :]).astype(f)
    hconst = np.einsum("buw,uh->bwh", adj * umask[None, :, :], C)  # [B,20,501]

    def aug(wT, brow):
        return np.concatenate([wT, brow[None, :]], axis=0).astype(f)

    def pad_rz(a):          # [s, 1002] -> [s, 1004] with per-gate 502 halves
        o = np.zeros((a.shape[0], 2 * HS2), dtype=f)
        o[:, :HS] = a[:, :HS]
        o[:, HS2:HS2 + HS] = a[:, HS:]
        return o

    def pad_h(a):           # [s, 501] -> [s, 502]
        o = np.zeros((a.shape[0], HS2), dtype=f)
        o[:, :HS] = a
        return o

    wrzh = pad_rz(aug(W_hh[:RZ].T, b_hh[:RZ]))
    whn = pad_h(aug(W_hh[RZ:].T, b_hh[RZ:]))
    wrzx = pad_rz(aug(W_ih[:RZ].T, b_ih[:RZ]))
    wxn = pad_h(aug(W_ih[RZ:].T, b_ih[RZ:]))
    wg = pad_h(np.concatenate([Wg[:, :HS].T, bg[None, :]], axis=0).astype(f))
    wgv = pad_h(np.ascontiguousarray(Wg[:, HS:].T))
    wm = pad_h(np.concatenate([Wm[:, :HS].T, np.zeros((1, HS), f)], axis=0))
    wmv = pad_h(np.ascontiguousarray(Wm[:, HS:].T))
    eye20 = np.eye(MAXN, dtype=f)
    w12 = np.concatenate([np.concatenate([W1.T, W2.T], axis=1),
                          np.concatenate([b1, b2])[None, :]], axis=0).astype(f)

    ents, ncolsf = _packf_layout()

    def place(pack, name, arr):
        nr, c0, ncl = ents[name]
        assert arr.shape == (nr, ncl), (name, arr.shape, (nr, ncl))
        pack[0:nr, c0:c0 + ncl] = arr

    in_maps = []
    for c in range(NCORES):
        sli = slice(c * bs, (c + 1) * bs)
        Xc = X[sli]                                   # [bs, 20, 48]
        xt = Xc.transpose(2, 1, 0).reshape(XD, MAXN * bs)
        adjc = adj[sli]                               # [bs, 20, 20]

        pack = np.zeros((128, ncolsf), dtype=f)
        place(pack, "xub", xt)
        for i, (o, s) in enumerate(CH):
            place(pack, f"wrzh{i}", wrzh[o:o + s])
            place(pack, f"whn{i}", whn[o:o + s])
            place(pack, f"w12{i}", w12[o:o + s])
            place(pack, f"wg{i}", wg[o:o + s])
            place(pack, f"wm{i}", wm[o:o + s])
        place(pack, "wrzx", wrzx)
        place(pack, "wxn", wxn)
        place(pack, "wgv", wgv)
        place(pack, "wmv", wmv)
        place(pack, "eye20", eye20)
        adjg = adjc.reshape(bs, MAXN * MAXN)
        place(pack, "adjg0", adjg[:128])
        place(pack, "adjg1", adjg[128:])

        bpack = np.zeros((128, NBCOLS), dtype=bf16)
        hcc = hconst[sli]                             # [bs, 20, 501]
        for t in range(2):
            for w in range(MAXN):
                col = (t * MAXN + w) * HS2
                bpack[:, col:col + HS] = hcc[t * 128:(t + 1) * 128, w, :]
                bpack[:, col + HS] = 1.0
        bpack[:, 40 * HS2:40 * HS2 + 128] = np.eye(128, dtype=f)

        in_maps.append(dict(wpack=pack, bpack=bpack))
    return in_maps


def _get_prog():
    global _PROG
    if _PROG is None:
        _PROG = _build_program()
    return _PROG


def kernel(**inputs):
    from concourse.bass_utils import run_bass_kernel_spmd
    nc = _get_prog()
    in_maps = _host_prep(**inputs)
    res = run_bass_kernel_spmd(nc, in_maps, core_ids=list(range(NCORES)))
    out = np.concatenate([r["out"] for r in res.results], axis=0)
    mu = np.ascontiguousarray(out[:, :NZ])
    logvar = np.ascontiguousarray(out[:, NZ:])
    return mu, logvar
